# revision 27
# baseline (speedup 1.0000x reference)
"""Self-contained Trainium2 Bass kernel: 16-head self-attention (B=4, N=2048,
C=1024, fp32), SPMD across 8 NeuronCores.

Entry point: kernel(**inputs) -> np.ndarray matching the reference module
(qkv projection + scaled-dot-product softmax attention + output projection).

Per-core sharding: core = (batch b = core//2, head-group hg = core%2); each
core computes 8 heads of one batch plus a partial output projection; the two
head-group partials per batch are summed on the host.

Kernel design:
  - all matmul operands bf16 (inputs cast on host); PSUM accumulation fp32.
    All stationaries are full 128x128 tiles (partial/quadrant tiles measure
    ~55ns/matmul slower), so kTp keeps the partner head's rows zeroed and
    v_sb blocks are zero-padded to 128 cols (zero-fill on the idle DVE).
  - the PE p-state ramp makes gaps cost ~2.5x their length (216ns/matmul
    sustained, ~427ns for ~3us after any pause), so emission is organized
    around a never-stalling in-order PE queue:
      * k/q pair 0 first, so exp (Scalar engine, ~267us busy, the
        second-highest engine) starts ~20us in;
      * remaining qkv packets are deadline-scheduled filler inside later
        heads' m-loops; v is computed inside head 0's m-loop;
      * attn@V accumulations trail the exp stream through a global queue
        (lag 4) and carry across head boundaries so the last chunks never
        wait on the Scalar engine;
      * the softmax divide chain (denominator reciprocal + ones-matmul
        broadcast + aoT multiply) is deferred one head;
      * nh0's output projection interleaves between nh1's heads; the final
        8 projection units run in waves with the g=3 contraction step
        deferred so the last head's divide chain hides under g=0..2.
  - softmax denominator rides as a ones-column inside the v_sb stationary
    (attn@V output partition 64); reciprocal via DMA-reshape to [128,8].

PSUM discipline (8 banks = two [128,1024]x2 pools):
  psA tag "sc": score tiles, v/kq filler tiles, bc broadcast, nh1-interleaved
    proj units -- transient tiles whose consumers (ACT/DVE/GPSIMD) always
    progress -> no deadlock against the in-order PE queue.
  psB tag "av": one long-lived attn@V accumulator per head; the final proj
    waves borrow both pools, keeping one slot free for the last bc tile.
"""
import numpy as np
import ml_dtypes

_NC_CACHE = {}


def kernel(x, Wqkv, bqkv, Wproj, bproj):
    from concourse.bass_utils import run_bass_kernel_spmd
    x = np.asarray(x, dtype=np.float32)
    Wqkv = np.asarray(Wqkv, dtype=np.float32)
    bqkv = np.asarray(bqkv, dtype=np.float32)
    Wproj = np.asarray(Wproj, dtype=np.float32)
    bproj = np.asarray(bproj, dtype=np.float32)
    wb = bool(np.any(bqkv) or np.any(bproj))
    if wb not in _NC_CACHE:
        nc = build_nc(with_biases=wb)
        split_excess_waits(nc)
        _NC_CACHE[wb] = nc
    nc = _NC_CACHE[wb]
    in_maps = shard_inputs(x, Wqkv, bqkv, Wproj, bproj)
    res = run_bass_kernel_spmd(nc, in_maps, core_ids=list(range(N_CORES)))
    return unshard_output(res.results).astype(np.float32)


# ======================================================================
# IR post-pass: this walrus build accepts at most one semaphore wait per
# ctrl instruction; overflow waits move onto chained NoOps just before the
# instruction on the same engine queue.
# ======================================================================

CTRL_OPCODES = {"Drain", "NoOp", "EventSemaphore", "AllEngineBarrier"}

def split_excess_waits(nc, engine_max=1, ctrl_max=1):
    n_split = 0
    for f in nc.m.functions:
        for bb in f.blocks:
            insts = list(bb.instructions)
            out = []
            changed = False
            for inst in insts:
                si = inst.sync_info
                max_w = ctrl_max if inst.opcode in CTRL_OPCODES else engine_max
                if si is not None and si.on_wait and len(si.on_wait) > max_w:
                    waits = list(si.on_wait)
                    extra, keep = waits[max_w:], waits[:max_w]
                    for i in range(0, len(extra), ctrl_max):
                        nop = bass_rust.InstNoOp(
                            name=f"{inst.name}-wsplit{i}", ins=[], outs=[])
                        nop.engine = inst.engine
                        nop.sync_info = mybir.SyncInfo(
                            on_wait=extra[i:i + ctrl_max], on_update=[])
                        out.append(nop)
                        n_split += 1
                    inst.sync_info = mybir.SyncInfo(
                        on_wait=keep, on_update=list(si.on_update))
                    changed = True
                out.append(inst)
            if changed:
                bb.instructions = out
    return n_split


# ======================================================================
# Kernel proper
# ======================================================================
import bass_rust
import concourse.bass as bass
import concourse.tile as tile
import concourse.mybir as mybir


F32 = mybir.dt.float32
F32R = mybir.dt.float32r
BF16 = mybir.dt.bfloat16

N = 2048        # sequence length
C = 1024        # embed dim
HL = 8          # heads handled per core
D = 64          # head dim
SCALE = D ** -0.5
NHALF = N // 2
N_CORES = 8

AFT = mybir.ActivationFunctionType


def build_nc(with_biases=False):
    nc = bass.Bass("TRN2", target_bir_lowering=False, debug=False,
                   num_devices=N_CORES)
    xt = nc.dram_tensor("xt", [C, N], BF16, kind="ExternalInput").ap()
    wq = nc.dram_tensor("wq", [C, HL * D], BF16, kind="ExternalInput").ap()
    wk = nc.dram_tensor("wk", [C, HL * D], BF16, kind="ExternalInput").ap()
    wv = nc.dram_tensor("wv", [C, HL * D], BF16, kind="ExternalInput").ap()
    wp = nc.dram_tensor("wp", [HL * D, C], BF16, kind="ExternalInput").ap()
    bqc = nc.dram_tensor("bqc", [128, 4], F32, kind="ExternalInput").ap()
    bkc = nc.dram_tensor("bkc", [128, 4], F32, kind="ExternalInput").ap()
    bv = nc.dram_tensor("bv", [1, HL * D], F32R, kind="ExternalInput").ap()
    bp = nc.dram_tensor("bp", [1, C], F32R, kind="ExternalInput").ap()
    ones_row = nc.dram_tensor("ones_row", [1, 512], F32R,
                              kind="ExternalInput").ap()
    out = nc.dram_tensor("out", [N, C], BF16, kind="ExternalOutput").ap()

    with tile.TileContext(nc) as tc:
        with tc.tile_pool(name="consts", bufs=1) as consts, \
             tc.tile_pool(name="persist", bufs=1) as persist, \
             tc.tile_pool(name="xpool", bufs=2) as xpool, \
             tc.tile_pool(name="expp", bufs=8) as expp, \
             tc.tile_pool(name="avsp", bufs=2) as avsp, \
             tc.tile_pool(name="rrp", bufs=2) as rrp, \
             tc.tile_pool(name="pout", bufs=3) as pout, \
             tc.tile_pool(name="psA", bufs=2, space="PSUM") as psA, \
             tc.tile_pool(name="psB", bufs=2, space="PSUM") as psB:

            ones = consts.tile([1, 512], F32R, tag="ones")
            bqc_sb = consts.tile([128, 4], F32, tag="bqc")
            bkc_sb = consts.tile([128, 4], F32, tag="bkc")
            if with_biases:
                bv_sb = consts.tile([1, HL * D], F32R, tag="bv")
                bp_sb = consts.tile([1, C], F32R, tag="bp")

            # persistent activation tiles.  kTp: per-head [128, h*N] blocks
            # with the partner head's 64 partition rows zeroed (full 128x128
            # stationaries run ~55ns/matmul faster than K=64 quadrant tiles).
            # v_sb: per (m-chunk, head) [128, 128] block: 64 v-cols, a ones
            # col (softmax denominator), 63 zero cols.
            qT = persist.tile([128, 4 * N], BF16, tag="qT")
            kTp = persist.tile([128, HL * N], BF16, tag="kTp")
            v_sb = persist.tile([128, 16 * HL * 128], BF16, tag="v")
            aoT = persist.tile([128, 4 * N], BF16, tag="aoT")
            kview = kTp.rearrange("p (h4 j n) -> p h4 j n", j=2, n=N)
            nc.vector.memset(kview[D:128, :, 0, :], 0.0)
            nc.vector.memset(kview[0:D, :, 1, :], 0.0)
            vview = v_sb.rearrange("p (m h x) -> p m h x", h=HL, x=128)
            nc.vector.memset(vview[:, :, :, D + 1:128], 0.0)
            nc.vector.memset(vview[:, :, :, D:D + 1], 1.0)


            # weight tiles: [128 (C-chunk partition), chunk x cols]
            wq_sb = consts.tile([128, 8 * 512], BF16, tag="wq")
            wk_sb = consts.tile([128, 8 * 512], BF16, tag="wk")
            wv_sb = consts.tile([128, 8 * 512], BF16, tag="wv")
            wp_sb = consts.tile([128, 4 * C], BF16, tag="wp")
            xT = [xpool.tile([128, 8 * NHALF], BF16, tag="xT",
                             name=f"xT{i}") for i in range(2)]

            # ---- input DMAs: each dma_start costs ~600ns of serial issue
            # time on its queue (DIRECT2D) and a queue moves ~40GB/s, so:
            # per-chunk granules for queue parallelism, split across TWO
            # issue queues (SP + Activation, which is idle until the first
            # exp at ~26us), in consumption order ----
            for c in range(8):
                nc.sync.dma_start(out=wk_sb[:, c * 512:(c + 1) * 512],
                                  in_=wk[c * 128:(c + 1) * 128, :])
                nc.sync.dma_start(
                    out=xT[0][:, c * NHALF:(c + 1) * NHALF],
                    in_=xt[c * 128:(c + 1) * 128, 0:NHALF])
            for c in range(8):
                nc.sync.dma_start(
                    out=xT[1][:, c * NHALF:(c + 1) * NHALF],
                    in_=xt[c * 128:(c + 1) * 128, NHALF:N])
                nc.sync.dma_start(out=wq_sb[:, c * 512:(c + 1) * 512],
                                  in_=wq[c * 128:(c + 1) * 128, :])
            nc.sync.dma_start(out=bqc_sb, in_=bqc)
            nc.sync.dma_start(out=bkc_sb, in_=bkc)
            for c in range(8):
                nc.sync.dma_start(out=wv_sb[:, c * 512:(c + 1) * 512],
                                  in_=wv[c * 128:(c + 1) * 128, :])
            for g in range(4):
                nc.sync.dma_start(out=wp_sb[:, g * C:(g + 1) * C],
                                  in_=wp[g * 128:(g + 1) * 128, :])
            nc.sync.dma_start(out=ones, in_=ones_row)
            if with_biases:
                nc.sync.dma_start(out=bv_sb, in_=bv)
                nc.sync.dma_start(out=bp_sb, in_=bp)

            # ---- emission helpers ----
            def kqps(dstT, w_sb, b_col, g, nh, pool):
                """One [128,1024] psum group: q or k pair g, n-half nh."""
                tag = "sc" if pool is psA else "av"
                ps = pool.tile([128, 1024], F32, tag=tag,
                               name=f"kq{g}_{nh}_{0 if dstT is qT else 1}")
                for c in range(8):
                    for ngl in range(2):
                        nc.tensor.matmul(
                            ps[:, ngl * 512:(ngl + 1) * 512],
                            w_sb[:, c * 512 + g * 128: c * 512 + (g + 1) * 128],
                            xT[nh][:, c * NHALF + ngl * 512:
                                   c * NHALF + ngl * 512 + 512],
                            start=(c == 0), stop=(c == 7))
                if dstT is qT:
                    nc.vector.tensor_scalar_add(
                        qT[:, g * N + nh * NHALF: g * N + nh * NHALF + 1024],
                        ps, b_col[:, g:g + 1])
                else:
                    n0 = nh * NHALF
                    for hh in range(2):
                        h_, r0_ = 2 * g + hh, hh * D
                        nc.vector.tensor_scalar_add(
                            kTp[r0_:r0_ + D, h_ * N + n0: h_ * N + n0 + 1024],
                            ps[r0_:r0_ + D, :], b_col[r0_:r0_ + D, g:g + 1])

            def vps(mp, pool):
                """v for m-chunk pair (2mp, 2mp+1) -> v_sb blocks."""
                nh, ml = (0, mp) if mp < 4 else (1, mp - 4)
                tag = "sc" if pool is psA else "av"
                ps = pool.tile([128, 1024], F32, tag=tag, name=f"v{mp}")
                for c in range(8):
                    for j in range(2):
                        nc.tensor.matmul(
                            ps[:, j * 512:(j + 1) * 512],
                            xT[nh][:, c * NHALF + (2 * ml + j) * 128:
                                   c * NHALF + (2 * ml + j + 1) * 128],
                            wv_sb[:, c * 512:(c + 1) * 512],
                            start=(c == 0),
                            stop=(c == 7 and not with_biases))
                for j in range(2):
                    if with_biases:
                        nc.tensor.matmul(ps[:, j * 512:(j + 1) * 512],
                                         ones[0:1, 0:128], bv_sb[0:1, :],
                                         start=False, stop=True)
                    mc = 2 * mp + j
                    nc.vector.tensor_copy(
                        vview[:, mc, :, 0:D],
                        ps[:, j * 512:(j + 1) * 512].rearrange(
                            "p (h e) -> p h e", e=D))

            fillers = []   # (deadline_key, closure); key = nh*8 + h

            def flush_fillers(key):
                while fillers and fillers[0][0] <= key:
                    fillers.pop(0)[1]()

            def pop_filler():
                if fillers:
                    fillers.pop(0)[1]()

            tails = []
            av_queue = []   # (av_acc_closure, finalize_or_None)

            def drain_av(target_len):
                while len(av_queue) > target_len:
                    fn, fin = av_queue.pop(0)
                    fn()
                    if fin is not None:
                        fin()

            def emit_tail(h, nh, avs, rrow):
                g, r0 = h // 2, (h % 2) * D
                n0 = nh * NHALF

                def tail(pool=None, tag=None):
                    pool, tag = pool or psA, tag or "sc"
                    bc = pool.tile([128, NHALF], F32, tag=tag,
                                   name=f"bc{h}_{nh}")
                    for ngl in range(2):
                        nc.tensor.matmul(
                            bc[0:D, ngl * 512:(ngl + 1) * 512],
                            ones[0:1, 0:D],
                            rrow[0:1, ngl * 512:(ngl + 1) * 512],
                            start=True, stop=True)
                    nc.vector.tensor_mul(
                        aoT[r0:r0 + D, g * N + n0: g * N + n0 + NHALF],
                        avs[0:D, :], bc[0:D, :])
                tails.append((nh, tail))
                if len(tails) > 1:
                    tails.pop(0)[1]()

            def head_block(h, nh, pacer=None, v_interleave=False,
                           av_lag=4):
                """Scores + exp + attn@V for head h (0..7) on n-half nh."""
                g, r0 = h // 2, (h % 2) * D
                n0 = nh * NHALF
                av = psB.tile([128, NHALF], F32, tag="av",
                              name=f"av{h}_{nh}")

                def av_acc(mcc, ex):
                    for ngl in range(2):
                        nc.tensor.matmul(
                            av[:, ngl * 512:(ngl + 1) * 512],
                            v_sb[:, (mcc * HL + h) * 128:
                                 (mcc * HL + h) * 128 + 128],
                            ex[:, ngl * 512:(ngl + 1) * 512],
                            start=(mcc == 0), stop=(mcc == 15))

                def finalize(av=av, h=h, nh=nh):
                    # denominator chain (consumed by the deferred tail):
                    # DVE recip is ~6 cyc/elem along the free dim, so reshape
                    # [1,1024] -> [128,8] via sbuf-to-sbuf DMA, recip across
                    # partitions, reshape back.  The last head reads the av
                    # accumulator directly (nothing reuses its slot, and the
                    # avs-copy hop would lengthen the kernel's tail chain).
                    avs = avsp.tile([D + 1, NHALF], F32, tag="avs",
                                    name=f"avs{h}_{nh}")
                    nc.vector.tensor_copy(avs, av[0:D + 1, :])
                    eng = nc.sync if (h == 7 and nh == 1) else nc.gpsimd
                    den = rrp.tile([128, NHALF // 128], F32R, tag="den",
                                   name=f"den{h}_{nh}")
                    eng.dma_start(out=den,
                                  in_=avs[D:D + 1, :].bitcast(F32R))
                    rcp = rrp.tile([128, NHALF // 128], F32R, tag="rcp",
                                   name=f"rcp{h}_{nh}")
                    with nc.allow_low_precision(reason="softmax denom"):
                        nc.vector.reciprocal(rcp, den)
                    rrow = rrp.tile([1, NHALF], F32R, tag="rr",
                                    name=f"rr{h}_{nh}")
                    eng.dma_start(out=rrow, in_=rcp)
                    emit_tail(h, nh, avs, rrow)

                for mcc in range(16):
                    # av-drain BEFORE the score pair: the sc tile below waits
                    # for an exp completion (slot backpressure against the
                    # saturated Scalar engine); the av matmuls have no slot
                    # dependency and fill exactly that wait window
                    drain_av(av_lag)
                    sc = psA.tile([128, NHALF], F32, tag="sc",
                                  name=f"sc{h}_{nh}_{mcc}")
                    for ngl in range(2):
                        nc.tensor.matmul(
                            sc[:, ngl * 512:(ngl + 1) * 512],
                            kTp[:, h * N + mcc * 128: h * N + (mcc + 1) * 128],
                            qT[:, g * N + n0 + ngl * 512:
                               g * N + n0 + (ngl + 1) * 512],
                            start=True, stop=True)
                    ex = expp.tile([128, NHALF], BF16, tag="ex",
                                   name=f"ex{h}_{nh}_{mcc}")
                    nc.scalar.activation(ex, sc, AFT.Exp, scale=SCALE)
                    av_queue.append((
                        lambda mcc=mcc, ex=ex: av_acc(mcc, ex),
                        finalize if mcc == 15 else None))
                    if v_interleave and mcc % 2 == 1:
                        vps(mcc // 2, psA)
                    if pacer is not None and mcc % 6 == 5:
                        pacer()

            def proj_mms(ps, nch, gs):
                """Projection matmuls for n-chunk nch over contraction
                groups gs (subset of 0..3, ending at 3)."""
                for jg in range(2):
                    for g in gs:
                        nc.tensor.matmul(
                            ps[:, jg * 512:(jg + 1) * 512],
                            aoT[:, g * N + nch * 128: g * N + (nch + 1) * 128],
                            wp_sb[:, g * C + jg * 512: g * C + jg * 512 + 512],
                            start=(g == 0),
                            stop=(g == 3 and not with_biases))
                    if 3 in gs and with_biases:
                        nc.tensor.matmul(
                            ps[:, jg * 512:(jg + 1) * 512], ones[0:1, 0:128],
                            bp_sb[0:1, jg * 512:(jg + 1) * 512],
                            start=False, stop=True)

            def proj_evac(ps, nch, last=False):
                po = pout.tile([128, 1024], BF16, tag="po", name=f"po{nch}")
                nc.vector.tensor_copy(po, ps)
                nsplit = 2 if last else 1
                rows = 128 // nsplit
                for s in range(nsplit):
                    nc.gpsimd.dma_start(
                        out=out[nch * 128 + s * rows:
                                nch * 128 + (s + 1) * rows, :],
                        in_=po[s * rows:(s + 1) * rows, :])

            def proj_unit(nch, last=False):
                """Full output projection for one n-chunk, from the transient
                psA pool (emitted between attention heads)."""
                ps = psA.tile([128, 1024], F32, tag="sc", name=f"pj{nch}")
                proj_mms(ps, nch, range(4))
                proj_evac(ps, nch, last)

            # ---- phase A head: k pair 0 (nh0) + q pair 0 unblock the first
            # 8 score chunks; k pair 0 (nh1) rides as the first filler inside
            # head 0's m-loop (its xT1 data lands ~25us in) ----
            kqps(kTp, wk_sb, bkc_sb, 0, 0, psA)
            kqps(qT, wq_sb, bqc_sb, 0, 0, psA)
            fillers.append((0, lambda: kqps(kTp, wk_sb, bkc_sb, 0, 1, psA)))

            # filler packets with deadlines (key = nh*8 + h of first consumer)
            for g in range(1, 4):
                fillers.append((2 * g, lambda g=g: kqps(kTp, wk_sb, bkc_sb,
                                                        g, 0, psA)))
                fillers.append((2 * g, lambda g=g: kqps(kTp, wk_sb, bkc_sb,
                                                        g, 1, psA)))
                fillers.append((2 * g, lambda g=g: kqps(qT, wq_sb, bqc_sb,
                                                        g, 0, psA)))
            for g in range(4):
                fillers.append((8 + 2 * g, lambda g=g: kqps(qT, wq_sb, bqc_sb,
                                                            g, 1, psA)))

            # ---- attention nh=0 ----
            # head 0 carries the v computation interleaved in its m-loop;
            # keep 2 filler packets in reserve for the nh transition
            def pop_reserved():
                if len(fillers) > 6:
                    fillers.pop(0)[1]()

            head_block(0, 0, v_interleave=True, pacer=pop_reserved)
            for h in range(1, 8):
                flush_fillers(h)
                head_block(h, 0, pacer=pop_reserved)

            # ---- attention nh=1 with proj(nh0) between heads ----
            for h in range(8):
                flush_fillers(8 + h)
                head_block(h, 1, pacer=pop_filler)
                # ALL nh0 tails must precede nh0 proj units (their aoT
                # writes are the proj stationaries)
                while tails and tails[0][0] == 0:
                    tails.pop(0)[1]()
                proj_unit(h)              # nh0 n-chunks 0..7
            drain_av(0)
            # ---- final 8 proj units in waves (psA+psB slots), g=3 deferred
            # so the last head's divide chain overlaps g=0..2.  Wave 0 takes
            # only 3 slots: the 4th stays free for the last tail's bc
            # broadcast (a 4-slot wave deadlocks against the tail chain). ----
            waves = [(8, 9, 10), (11, 12, 13), (14, 15)]
            slots = [(psA, "sc"), (psA, "sc"), (psB, "av"), (psB, "av")]
            first = True
            for wave in waves:
                units = []
                for i, nch in enumerate(wave):
                    pool, tag = slots[i]
                    ps = pool.tile([128, 1024], F32, tag=tag,
                                   name=f"pj{nch}")
                    units.append((ps, nch))
                    proj_mms(ps, nch, range(3))
                if first:
                    while tails:
                        tails.pop(0)[1](psB, "av")
                    first = False
                for ps, nch in units:
                    proj_mms(ps, nch, [3])
                for ps, nch in units:
                    proj_evac(ps, nch, last=(nch == 15))
    return nc


def shard_inputs(x, Wqkv, bqkv, Wproj, bproj):
    """Full inputs -> per-core in_maps. Core c: batch c//2, head-group c%2."""
    bf = ml_dtypes.bfloat16
    in_maps = []
    for core in range(N_CORES):
        b, hg = core // 2, core % 2
        s = hg * 512
        m = {
            "xt": np.ascontiguousarray(x[b].T.astype(bf)),
            "wq": np.ascontiguousarray(Wqkv[:, s:s + 512].astype(bf)),
            "wk": np.ascontiguousarray(Wqkv[:, C + s: C + s + 512].astype(bf)),
            "wv": np.ascontiguousarray(
                Wqkv[:, 2 * C + s: 2 * C + s + 512].astype(bf)),
            "wp": np.ascontiguousarray(Wproj[s:s + 512, :].astype(bf)),
            "bqc": np.ascontiguousarray(bqkv[s:s + 512].reshape(4, 128).T),
            "bkc": np.ascontiguousarray(
                bqkv[C + s: C + s + 512].reshape(4, 128).T),
            "bv": np.ascontiguousarray(bqkv[2 * C + s: 2 * C + s + 512][None, :]),
            "bp": np.ascontiguousarray(
                (bproj if hg == 0 else np.zeros_like(bproj))[None, :]),
            "ones_row": np.ones((1, 512), np.float32),
        }
        in_maps.append(m)
    return in_maps


def unshard_output(results):
    """Per-core partial outputs -> full [4, N, C]."""
    outs = []
    for b in range(4):
        outs.append(results[2 * b]["out"].astype(np.float32) +
                    results[2 * b + 1]["out"].astype(np.float32))
    return np.stack(outs, axis=0)
